# revision 25
# baseline (speedup 1.0000x reference)
"""Causal self-attention on 8 NeuronCores (Trainium2, Bass/Tile).

Sharding: core c handles batch b = c//2 and head-group hg = c%2
(8 of 16 heads = 512 of 1024 feature dims). W_qkv is split column-wise,
W_proj row-wise per head group; each core returns a partial [T, D]
projection output (bf16) and the host sums the two partials per batch.

Per-core dataflow:
  The q/k projections run as DoubleRow fp8e4m3 matmuls — two 128-deep
  K-chunks per pass, 2x the bf16 column rate — with x pre-scaled by 8
  and Wq/Wk by 16 (the 1/16384 correction is folded into the exp()
  scale). Softmax averaging suppresses the fp8 score noise. v must stay
  bf16: the early causal rows attend to a handful of keys, so v
  quantization error passes straight through to the output.
    qT/kT = Wq/Wk.T @ x.T   [512, 2048]  (head-dim major, bf16 result)
    v     = x @ Wv          [2048, 512]  (bf16; + ones col per head)
  Attention proper stays bf16:
    sT[j,i] = kT.T @ qT     per head, causal-skipped/shrunk tiles
    PT = exp(SCALE8 * sT) (*mask on diagonal strips)
    UT[e,i], denom[i] = [v|1].T @ PT    (ones col -> denom)
    affinT = UT * (1/denom)  broadcast via K=8 bf16 matmul with E matrix
    partial = affinT.T @ Wp  accumulated over e-chunks, DMA'd out bf16
DMA dispatch is spread over the three DGE-capable engines (Sync ~650ns
per dispatch is a serial resource): x8 on Scalar, filler weights and the
dripped outA stores on GpSimd, everything latency-critical (denominator
repacks, outB stores) on the otherwise-quiet Sync.
"""

import sys

for _p in ("/opt/trn_rl_repo",):
    if _p not in sys.path:
        sys.path.append(_p)

import ml_dtypes
import numpy as np

import concourse.bass as bass
import concourse.tile as tile
from concourse import bacc, mybir
from concourse.bass_utils import run_bass_kernel_spmd

F32 = mybir.dt.float32
BF16 = mybir.dt.bfloat16
FP8 = mybir.dt.float8e4
DR = mybir.MatmulPerfMode.DoubleRow
EXP = mybir.ActivationFunctionType.Exp

B, T, D = 4, 2048, 1024
H, Dh = 16, 64
SCALE = float(D) ** -0.5
NCORES = 8
DL = 512          # local (per-core) feature width = 8 heads * 64
HL = 8            # local heads
NDC = D // 128    # 8 d-chunks
NPAIR = NDC // 2  # 4 DoubleRow d-chunk pairs
NEC = DL // 128   # 4 e-chunks (head pairs)
NTB = T // 512    # 4 t-blocks of 512
NTC = T // 128    # 16 t-chunks of 128
VPAIR = 192       # v_sb per-pair block: [v_even(64) | one | junk(63) | v_odd(64)]
VROW = NEC * VPAIR  # 640 cols per v_sb tile
XS, WS = 8.0, 16.0          # fp8 pre-scales for x and W_{q,k,v}
SCALE8 = SCALE / ((XS * WS) ** 2)


def _build():
    nc = bacc.Bacc("TRN2", target_bir_lowering=False, debug=False,
                   num_devices=NCORES)

    x8 = nc.declare_dram_parameter("x8", [DL, 2 * T], FP8, isOutput=False)
    wq8 = nc.declare_dram_parameter("wq8", [DL, 2 * DL], FP8, isOutput=False)
    wk8 = nc.declare_dram_parameter("wk8", [DL, 2 * DL], FP8, isOutput=False)
    xT = nc.declare_dram_parameter("xT", [D, T], BF16, isOutput=False)
    wv = nc.declare_dram_parameter("wv", [D, DL], BF16, isOutput=False)
    wp = nc.declare_dram_parameter("wp", [DL, D], BF16, isOutput=False)
    mask = nc.declare_dram_parameter("mask", [128, 128], BF16, isOutput=False)
    emat = nc.declare_dram_parameter("emat", [HL, DL], BF16, isOutput=False)
    outA = nc.declare_dram_parameter("outA", [T, D], BF16, isOutput=True)
    outB = nc.declare_dram_parameter("outB", [T, D], BF16, isOutput=True)

    with tile.TileContext(nc) as tc:
        _emit(nc, tc, x8, wq8, wk8, xT, wv, wp, mask, emat, outA, outB)
    nc.compile()
    return nc


def _emit(nc, tc, x8, wq8, wk8, xT, wv, wp, mask, emat, outA, outB):
    from contextlib import ExitStack

    ctx = ExitStack()
    with ctx:
        wqk_pool = ctx.enter_context(tc.tile_pool(name="wqk", bufs=16))
        qk_pool = ctx.enter_context(tc.tile_pool(name="qk", bufs=9))
        vsb_pool = ctx.enter_context(tc.tile_pool(name="vsb", bufs=NTC))
        ut_pool = ctx.enter_context(tc.tile_pool(name="ut", bufs=NEC))
        dn_pool = ctx.enter_context(tc.tile_pool(name="dn", bufs=1))
        dns_pool = ctx.enter_context(tc.tile_pool(name="dns", bufs=2))
        pt_pool = ctx.enter_context(tc.tile_pool(name="pt", bufs=4))
        cst_pool = ctx.enter_context(tc.tile_pool(name="cst", bufs=1))
        wp_pool = ctx.enter_context(tc.tile_pool(name="wp", bufs=NEC))
        stage_pool = ctx.enter_context(tc.tile_pool(name="stage", bufs=4))
        ps_pool = ctx.enter_context(tc.tile_pool(name="ps", bufs=1, space="PSUM"))

        # constants
        mk_sb = cst_pool.tile([128, 128], BF16, tag="mk")
        nc.sync.dma_start(mk_sb[:], mask[:])
        em_sb = cst_pool.tile([HL, DL], BF16, tag="em")
        nc.sync.dma_start(em_sb[:], emat[:])

        # PE warm-up: ~6us of dependency-free matmuls on a zeroed tile while
        # the input DMAs stream in. HAM un-throttles the PE clock (1.2 ->
        # 2.4 GHz) after ~3.4us of sustained activity; without this the whole
        # DMA-paced v phase runs at half clock.
        wu_sb = cst_pool.tile([128, 512], BF16, tag="wu")
        nc.gpsimd.memset(wu_sb[:], 0.0)
        for _ in range(28):
            ps_w = ps_pool.tile([128, 512], F32, tag="qkps", name="ps_qkps",
                                bufs=2)
            nc.tensor.matmul(ps_w[:], wu_sb[:, 0:128], wu_sb[:],
                             start=True, stop=True)

        # persistent tiles
        ut_sb = [ut_pool.tile([128, T], BF16, tag="ut", name=f"ut{i}")
                 for i in range(NEC)]
        dn_sb = dn_pool.tile([HL, T], F32, tag="dn")
        rd_sb = dn_pool.tile([HL, T], F32, tag="rd")
        rdb_sb = dn_pool.tile([HL, T], BF16, tag="rdb")
        v_sb = [vsb_pool.tile([128, VROW], BF16, tag="vsb", name=f"vsb{i}")
                for i in range(NTC)]
        # garbage rows of dn would hit reciprocal before they are written;
        # keep them finite so 0*inf NaNs can't leak out of the R matmul
        nc.gpsimd.memset(dn_sb[:], 1.0)

        wp_sb = []
        for ecn in range(NEC):
            t = wp_pool.tile([128, D], BF16, tag="wp", name=f"wpt{ecn}")
            nc.gpsimd.dma_start(t[:], wp[ecn * 128:(ecn + 1) * 128, :])
            wp_sb.append(t)

        def ps_tile(tag, bufs):
            return ps_pool.tile([128, 512], F32, tag=tag, name=f"ps_{tag}",
                                bufs=bufs)

        def proj_pass(ecs, out_t, dge):
            """One projection pass accumulating a subset of e-chunks into
            its own partial output (summed on the host)."""
            for tcn in range(NTC):
                for ob in range(2):
                    ps_p = ps_tile("qkps", 2)
                    for i, ecn in enumerate(ecs):
                        nc.tensor.matmul(
                            ps_p[:],
                            ut_sb[ecn][:, tcn * 128:(tcn + 1) * 128],
                            wp_sb[ecn][:, ob * 512:(ob + 1) * 512],
                            start=(i == 0), stop=(i == len(ecs) - 1))
                    st = stage_pool.tile([128, 512], BF16, tag="st",
                                         name="stg")
                    nc.vector.tensor_copy(st[:], ps_p[:])
                    dge.dma_start(
                        out_t[tcn * 128:(tcn + 1) * 128,
                              ob * 512:(ob + 1) * 512], st[:])
                    yield

        # projA drips during chunks 2-3 (outA stores ride gpsimd so the
        # quiet Sync queue keeps the denominator repacks low-latency);
        # projB's t-blocks 0-1 drip into the tail of chunk 3 right after
        # their normalize, the rest interleaves with normalize(3)
        projA = proj_pass((0, 1), outA, nc.gpsimd)
        projB = proj_pass((2, 3), outB, nc.sync)

        with tc.tile_pool(name="x8", bufs=NPAIR) as x8_pool:
            x8_sb = []
            qk_chunks = {}

            def pair3(t, free):
                """[128, 2*free] tile -> DoubleRow [K, 2, free] view."""
                return t[:].rearrange("p (two f) -> p two f", two=2)

            def qk_filler(ec):
                """Generator computing q/k chunks for `ec` via DoubleRow fp8
                matmuls; yields between small PE steps so it can be dripped
                into the attention loop as filler work. kT is stored twice,
                zero-padded per head parity, so the score matmuls run with
                K=128."""
                wq_t, wk_t = [], []
                for p in range(NPAIR):
                    t = wqk_pool.tile([128, 256], FP8, tag="wqk", name="wqkt")
                    nc.gpsimd.dma_start(
                        t[:], wq8[p * 128:(p + 1) * 128,
                                  ec * 256:(ec + 1) * 256])
                    wq_t.append(t)
                for p in range(NPAIR):
                    t = wqk_pool.tile([128, 256], FP8, tag="wqk", name="wqkt")
                    nc.gpsimd.dma_start(
                        t[:], wk8[p * 128:(p + 1) * 128,
                                  ec * 256:(ec + 1) * 256])
                    wk_t.append(t)
                q_ec = qk_pool.tile([128, T], BF16, tag="qk", name="q_ec")
                kA = qk_pool.tile([128, T], BF16, tag="qk", name="kA")
                kB = qk_pool.tile([128, T], BF16, tag="qk", name="kB")
                nc.gpsimd.memset(kA[64:128, :], 0.0)
                nc.gpsimd.memset(kB[0:64, :], 0.0)
                qk_chunks[ec] = (q_ec, kA, kB)
                for (w_t, iskA) in ((wq_t, False), (wk_t, True)):
                    for tbp in range(2):
                        pss = [ps_tile("qkps", 2) for _ in range(2)]
                        for p in range(NPAIR):
                            for i in range(2):
                                tb = 2 * tbp + i
                                nc.tensor.matmul(
                                    pss[i], pair3(w_t[p], 128),
                                    pair3(x8_sb[p], T)[
                                        :, :, tb * 512:(tb + 1) * 512],
                                    start=(p == 0), stop=(p == NPAIR - 1),
                                    perf_mode=DR)
                            yield
                        for i in range(2):
                            tb = 2 * tbp + i
                            sl = slice(tb * 512, (tb + 1) * 512)
                            if iskA:
                                nc.vector.tensor_copy(
                                    kA[0:64, sl], pss[i][0:64, :])
                                nc.vector.tensor_copy(
                                    kB[64:128, sl], pss[i][64:128, :])
                            else:
                                nc.vector.tensor_copy(q_ec[:, sl], pss[i][:])
                        yield

            def normalize(ec, tbs=None):
                """affinT = UT * 1/denom for chunk ec, sliced per t-block so
                the recip -> broadcast -> scale chain pipelines."""
                for tb in (range(NTB) if tbs is None else tbs):
                    sl = slice(tb * 512, (tb + 1) * 512)
                    nc.vector.reciprocal_approx_fast(rd_sb[:, sl],
                                                     dn_sb[:, sl])
                    nc.vector.tensor_copy(rdb_sb[:, sl], rd_sb[:, sl])
                    ps_r = ps_tile("qkps", 2)
                    nc.tensor.matmul(
                        ps_r[:], em_sb[:, ec * 128:(ec + 1) * 128],
                        rdb_sb[:, sl], start=True, stop=True)
                    nc.vector.tensor_mul(
                        ut_sb[ec][:, sl], ut_sb[ec][:, sl], ps_r[:])

            # ------------- phase A0: v = x @ Wv (+ dripped qk(0)) -------------
            with tc.tile_pool(name="wv", bufs=NDC) as wv_pool, \
                 tc.tile_pool(name="xt", bufs=NDC) as xt_pool:
                # xT and x8 stream in on the scalar engine's DGE queue so
                # their dispatches don't serialize behind wv/weights on
                # Sync; low-t halves of every chunk land first so the
                # dc-ordered accumulations can finish low-t blocks early
                wv_sb, xt_sb = [], []
                for dc in range(NDC):
                    t = wv_pool.tile([128, DL], BF16, tag="wv", name=f"wv{dc}")
                    nc.sync.dma_start(t[:], wv[dc * 128:(dc + 1) * 128, :])
                    wv_sb.append(t)
                    t = xt_pool.tile([128, T], BF16, tag="xt", name=f"xt{dc}")
                    nc.scalar.dma_start(t[:, 0:T // 2],
                                        xT[dc * 128:(dc + 1) * 128, 0:T // 2])
                    xt_sb.append(t)
                for p in range(NPAIR):
                    t = x8_pool.tile([128, 2 * T], FP8, tag="x8",
                                     name=f"x8{p}")
                    nc.scalar.dma_start(t[:, 0:T // 2],
                                        x8[p * 128:(p + 1) * 128, 0:T // 2])
                    nc.scalar.dma_start(
                        t[:, T:T + T // 2],
                        x8[p * 128:(p + 1) * 128, T:T + T // 2])
                    x8_sb.append(t)
                for dc in range(NDC):
                    nc.scalar.dma_start(
                        xt_sb[dc][:, T // 2:T],
                        xT[dc * 128:(dc + 1) * 128, T // 2:T])
                for p in range(NPAIR):
                    nc.scalar.dma_start(
                        x8_sb[p][:, T // 2:T],
                        x8[p * 128:(p + 1) * 128, T // 2:T])
                    nc.scalar.dma_start(
                        x8_sb[p][:, T + T // 2:2 * T],
                        x8[p * 128:(p + 1) * 128, T + T // 2:2 * T])

                filler0 = qk_filler(0)
                for tcn in range(NTC):
                    ps_v = ps_tile("utps", 2)
                    for dc in range(NDC):
                        nc.tensor.matmul(
                            ps_v[:], xt_sb[dc][:, tcn * 128:(tcn + 1) * 128],
                            wv_sb[dc][:], start=(dc == 0),
                            stop=(dc == NDC - 1))
                    dst = v_sb[tcn][:].rearrange("p (e c) -> p e c", c=VPAIR)
                    src = ps_v[:].rearrange("p (e c) -> p e c", c=128)
                    nc.vector.tensor_copy(dst[:, :, 0:64], src[:, :, 0:64])
                    nc.vector.tensor_copy(dst[:, :, 128:192], src[:, :, 64:128])
                    nc.gpsimd.memset(dst[:, :, 64:65], 1.0)
                    nc.gpsimd.memset(dst[:, :, 65:128], 0.0)
                    # hold the filler back until the x8 halves it reads have
                    # landed, so its matmuls never stall the PE ahead of
                    # ready v work
                    if tcn >= 4:
                        next(filler0, None)
                for _ in filler0:
                    pass
            # wv + xt pools released here (fillers stream x8, not xT)

            # ------------- per e-chunk: attention + dripped filler work -------------
            def attention_chunk(ec, drip):
                q_ec, kA, kB = qk_chunks.pop(ec)
                slot = [0]
                for par in range(2):       # head parity within chunk
                    h = 2 * ec + par       # local head index
                    kpad = kA if par == 0 else kB
                    for ibp in range(2):   # i-block pair (2*ibp, 2*ibp+1)
                        ibl, ibr = 2 * ibp, 2 * ibp + 1
                        utl = ps_tile("utps", 2)
                        utr = ps_tile("utps", 2)
                        for jt in range(4 * ibr + 4):
                            drip(slot[0])
                            slot[0] += 1
                            dl = (jt // 4 == ibl)
                            skip_l = (jt // 4 > ibl)
                            dr = (jt // 4 == ibr)
                            cl = 128 * (jt - 4 * ibl) if dl else 0
                            cr = 128 * (jt - 4 * ibr) if dr else 0
                            c0 = 512 + cr if skip_l else cl
                            st_ps = ps_pool.tile([128, 1024], F32, tag="stps",
                                                 name="ps_stps", bufs=2)
                            kh_j = kpad[:, jt * 128:(jt + 1) * 128]
                            if not skip_l:
                                nc.tensor.matmul(
                                    st_ps[:, cl:512], kh_j,
                                    q_ec[:, ibl * 512 + cl:(ibl + 1) * 512],
                                    start=True, stop=True)
                            nc.tensor.matmul(
                                st_ps[:, 512 + cr:1024], kh_j,
                                q_ec[:, ibr * 512 + cr:(ibr + 1) * 512],
                                start=True, stop=True)
                            pt_t = pt_pool.tile([128, 1024], BF16, tag="pt")
                            nc.scalar.activation(
                                pt_t[:, c0:1024], st_ps[:, c0:1024], EXP,
                                scale=SCALE8)
                            if dl:
                                nc.vector.tensor_mul(
                                    pt_t[:, cl:cl + 128],
                                    pt_t[:, cl:cl + 128], mk_sb[:])
                            if dr:
                                nc.vector.tensor_mul(
                                    pt_t[:, 512 + cr:512 + cr + 128],
                                    pt_t[:, 512 + cr:512 + cr + 128], mk_sb[:])
                            # PV: [v|1].T @ PT -> UT rows + denom row
                            vt = v_sb[jt][:].rearrange(
                                "p (e c) -> p e c", c=VPAIR)[:, ec, :]
                            if par == 0:
                                lhs = vt[:, 0:65]       # M=65 -> rows 0..64
                                rsl = slice(0, 65)
                            else:
                                # [one|junk63|v_odd]: denom row 0, v 64..127
                                lhs = vt[:, 64:192]     # M=128
                                rsl = slice(0, 128)
                            if not skip_l:
                                nc.tensor.matmul(
                                    utl[rsl, cl:512], lhs, pt_t[:, cl:512],
                                    start=(jt == 0), stop=(jt == 4 * ibl + 3),
                                    skip_group_check=True)
                            nc.tensor.matmul(
                                utr[rsl, cr:512], lhs, pt_t[:, 512 + cr:1024],
                                start=(jt == 0), stop=(jt == 4 * ibr + 3),
                                skip_group_check=True)
                            for ib_d, ut_d in ((ibl, utl), (ibr, utr)):
                                if jt != 4 * ib_d + 3:
                                    continue
                                if par == 0:
                                    usrc, dsrc, r = (ut_d[0:64, :],
                                                     ut_d[64:65, :], 64)
                                    udst = ut_sb[ec][
                                        0:64, ib_d * 512:(ib_d + 1) * 512]
                                else:
                                    usrc, dsrc, r = (ut_d[64:128, :],
                                                     ut_d[0:1, :], 0)
                                    udst = ut_sb[ec][
                                        64:128, ib_d * 512:(ib_d + 1) * 512]
                                with tc.high_priority():
                                    nc.vector.tensor_copy(udst, usrc)
                                    # denom: same-partition copy + DMA repack
                                    stg = dns_pool.tile([128, 512], F32,
                                                        tag="dns",
                                                        name="dnstg")
                                    nc.vector.tensor_copy(stg[r:r + 1, :],
                                                          dsrc)
                                nc.sync.dma_start(
                                    dn_sb[h:h + 1,
                                          ib_d * 512:(ib_d + 1) * 512],
                                    stg[r:r + 1, :])

            # The three remaining fillers (20 PE steps each) feed one shared
            # work stream dripped across chunks 0-2; pacing keeps each
            # filler comfortably ahead of the chunk that consumes it
            # (f1 by mid-c0, f2 by mid-c1, f3 by early c2) while leaving
            # every chunk enough PE drip work to bridge the exp latency.
            from itertools import chain as _chain
            fillers = _chain(qk_filler(1), qk_filler(2), qk_filler(3))

            for ec in range(NEC - 1):

                def drip(slot, ec=ec):
                    if ec < 2:
                        if slot % 8 in (0, 2, 3, 5):      # 24 per chunk
                            next(fillers, None)
                        elif ec > 0 and slot == 6:
                            normalize(ec - 1)
                    else:
                        if slot % 12 in (0, 3, 5, 8, 10):  # f3 done by ~s28
                            if next(fillers, None) is None and slot >= 9:
                                next(projA, None)
                        elif slot == 6:
                            normalize(ec - 1)
                        elif slot >= 9:
                            next(projA, None)

                attention_chunk(ec, drip)
                if ec == 2:
                    for _ in fillers:   # safety drain (normally empty)
                        pass
        # x8 pool released here (before the last attention chunk)

        def drip3(slot):
            if slot == 6:
                normalize(NEC - 2)
            elif slot == 29:
                normalize(NEC - 1, [0])
            elif slot == 33:
                normalize(NEC - 1, [1])
            elif slot >= 38:
                next(projB, None)
            elif slot != 0:
                next(projA, None)

        attention_chunk(NEC - 1, drip3)
        # 6 ready t-block-0/1 projB steps are held back from the chunk so
        # the PE has dependency-free work covering the two trailing
        # normalize DVE chains (else HAM re-throttles the clock here)
        for _ in projA:
            pass
        for _ in range(3):
            next(projB, None)
        normalize(NEC - 1, [2])
        for _ in range(3):
            next(projB, None)
        normalize(NEC - 1, [3])
        for _ in projB:              # t-blocks 2-3
            pass



_NC_CACHE = None


def _get_nc():
    global _NC_CACHE
    if _NC_CACHE is None:
        _NC_CACHE = _build()
    return _NC_CACHE


def make_in_maps(embds, W_qkv, W_proj):
    embds = np.asarray(embds, dtype=np.float32)
    W_qkv = np.asarray(W_qkv, dtype=np.float32)
    W_proj = np.asarray(W_proj, dtype=np.float32)
    E4 = ml_dtypes.float8_e4m3

    mask_np = np.triu(np.ones((128, 128))).astype(ml_dtypes.bfloat16)
    emat_np = np.kron(np.eye(HL), np.ones((1, Dh))).astype(ml_dtypes.bfloat16)

    def pack_w(w):
        # [1024, 512] -> [512, 1024]: row pair*128+r, col ec*256+two*128+c
        return np.ascontiguousarray(
            (WS * w).reshape(NPAIR, 2, 128, NEC, 128)
            .transpose(0, 2, 3, 1, 4).reshape(DL, 2 * DL)).astype(E4)

    in_maps = []
    for c in range(NCORES):
        b, hg = c // 2, c % 2
        sl = slice(hg * DL, (hg + 1) * DL)
        xT = np.ascontiguousarray(embds[b].T)
        x8 = np.ascontiguousarray(
            (XS * xT).reshape(NPAIR, 2, 128, T)
            .transpose(0, 2, 1, 3).reshape(DL, 2 * T)).astype(E4)
        in_maps.append({
            "x8": x8,
            "wk8": pack_w(W_qkv[:, 0 * D:1 * D][:, sl]),
            "wq8": pack_w(W_qkv[:, 1 * D:2 * D][:, sl]),
            "xT": xT.astype(ml_dtypes.bfloat16),
            "wv": np.ascontiguousarray(
                W_qkv[:, 2 * D:3 * D][:, sl]).astype(ml_dtypes.bfloat16),
            "wp": np.ascontiguousarray(
                W_proj[sl, :]).astype(ml_dtypes.bfloat16),
            "mask": mask_np,
            "emat": emat_np,
        })
    return in_maps


def gather_out(results, b_proj):
    b_proj = np.asarray(b_proj, dtype=np.float32)
    full = np.empty((B, T, D), dtype=np.float32)
    for b in range(B):
        acc = np.zeros((T, D), dtype=np.float32)
        for c in (2 * b, 2 * b + 1):
            acc += np.asarray(results[c]["outA"], dtype=np.float32)
            acc += np.asarray(results[c]["outB"], dtype=np.float32)
        full[b] = acc + b_proj[None, :]
    return full


def kernel(embds, W_qkv, W_proj, b_proj):
    in_maps = make_in_maps(embds, W_qkv, W_proj)
    nc = _get_nc()
    res = run_bass_kernel_spmd(nc, in_maps, list(range(NCORES)))
    return gather_out(res.results, b_proj)


# revision 33
# speedup vs baseline: 1.0179x; 1.0179x over previous
"""Causal self-attention on 8 NeuronCores (Trainium2, Bass/Tile).

Sharding: core c handles batch b = c//2 and head-group hg = c%2
(8 of 16 heads = 512 of 1024 feature dims). W_qkv is split column-wise,
W_proj row-wise per head group; each core returns a partial [T, D]
projection output (bf16) and the host sums the two partials per batch.

Per-core dataflow:
  The q/k projections run as DoubleRow fp8e4m3 matmuls — two 128-deep
  K-chunks per pass, 2x the bf16 column rate — with x pre-scaled by 8
  and Wq/Wk by 16 (the 1/16384 correction is folded into the exp()
  scale). Softmax averaging suppresses the fp8 score noise. v must stay
  bf16: the early causal rows attend to a handful of keys, so v
  quantization error passes straight through to the output.
    qT/kT = Wq/Wk.T @ x.T   [512, 2048]  (head-dim major, bf16 result)
    v     = x @ Wv          [2048, 512]  (bf16; + ones col per head)
  Attention proper stays bf16:
    sT[j,i] = kT.T @ qT     per head, causal-skipped/shrunk tiles
    PT = exp(SCALE8 * sT) (*mask on diagonal strips)
    UT[e,i], denom[i] = [v|1].T @ PT    (ones col -> denom)
    affinT = UT * (1/denom)  broadcast via K=8 bf16 matmul with E matrix
    partial = affinT.T @ Wp  accumulated over e-chunks, DMA'd out bf16
DMA dispatch is spread over the three DGE-capable engines (Sync ~650ns
per dispatch is a serial resource): x8 on Scalar, filler weights and the
dripped outA stores on GpSimd, everything latency-critical (denominator
repacks, outB stores) on the otherwise-quiet Sync.
"""

import sys

for _p in ("/opt/trn_rl_repo",):
    if _p not in sys.path:
        sys.path.append(_p)

import ml_dtypes
import numpy as np

import concourse.bass as bass
import concourse.tile as tile
from concourse import bacc, mybir
from concourse.bass_utils import run_bass_kernel_spmd

F32 = mybir.dt.float32
BF16 = mybir.dt.bfloat16
FP8 = mybir.dt.float8e4
DR = mybir.MatmulPerfMode.DoubleRow
EXP = mybir.ActivationFunctionType.Exp

B, T, D = 4, 2048, 1024
H, Dh = 16, 64
SCALE = float(D) ** -0.5
NCORES = 8
DL = 512          # local (per-core) feature width = 8 heads * 64
HL = 8            # local heads
NDC = D // 128    # 8 d-chunks
NPAIR = NDC // 2  # 4 DoubleRow d-chunk pairs
NEC = DL // 128   # 4 e-chunks (head pairs)
NTB = T // 512    # 4 t-blocks of 512
NTC = T // 128    # 16 t-chunks of 128
VPAIR = 192       # v_sb per-pair block: [v_even(64) | one | junk(63) | v_odd(64)]
VROW = NEC * VPAIR  # 640 cols per v_sb tile
XS, WS = 8.0, 16.0          # fp8 pre-scales for x and W_{q,k,v}
SCALE8 = SCALE / ((XS * WS) ** 2)


def _build():
    nc = bacc.Bacc("TRN2", target_bir_lowering=False, debug=False,
                   num_devices=NCORES)

    x8 = nc.declare_dram_parameter("x8", [DL, 2 * T], FP8, isOutput=False)
    wq8 = nc.declare_dram_parameter("wq8", [DL, 2 * DL], FP8, isOutput=False)
    wk8 = nc.declare_dram_parameter("wk8", [DL, 2 * DL], FP8, isOutput=False)
    xT = nc.declare_dram_parameter("xT", [D, T], BF16, isOutput=False)
    wv = nc.declare_dram_parameter("wv", [D, DL], BF16, isOutput=False)
    wp = nc.declare_dram_parameter("wp", [DL, D], BF16, isOutput=False)
    mask = nc.declare_dram_parameter("mask", [128, 128], BF16, isOutput=False)
    emat = nc.declare_dram_parameter("emat", [HL, DL], BF16, isOutput=False)
    outA = nc.declare_dram_parameter("outA", [T, D], BF16, isOutput=True)
    outB = nc.declare_dram_parameter("outB", [T, D], BF16, isOutput=True)

    with tile.TileContext(nc) as tc:
        _emit(nc, tc, x8, wq8, wk8, xT, wv, wp, mask, emat, outA, outB)
    nc.compile()
    return nc


def _emit(nc, tc, x8, wq8, wk8, xT, wv, wp, mask, emat, outA, outB):
    from contextlib import ExitStack

    ctx = ExitStack()
    with ctx:
        wqk_pool = ctx.enter_context(tc.tile_pool(name="wqk", bufs=16))
        qk_pool = ctx.enter_context(tc.tile_pool(name="qk", bufs=9))
        vsb_pool = ctx.enter_context(tc.tile_pool(name="vsb", bufs=NTC))
        ut_pool = ctx.enter_context(tc.tile_pool(name="ut", bufs=NEC))
        dn_pool = ctx.enter_context(tc.tile_pool(name="dn", bufs=1))
        dns_pool = ctx.enter_context(tc.tile_pool(name="dns", bufs=2))
        pt_pool = ctx.enter_context(tc.tile_pool(name="pt", bufs=4))
        cst_pool = ctx.enter_context(tc.tile_pool(name="cst", bufs=1))
        wp_pool = ctx.enter_context(tc.tile_pool(name="wp", bufs=NEC))
        stage_pool = ctx.enter_context(tc.tile_pool(name="stage", bufs=6))
        ps_pool = ctx.enter_context(tc.tile_pool(name="ps", bufs=1, space="PSUM"))

        # constants
        mk_sb = cst_pool.tile([128, 128], BF16, tag="mk")
        nc.sync.dma_start(mk_sb[:], mask[:])
        em_sb = cst_pool.tile([HL, DL], BF16, tag="em")
        nc.sync.dma_start(em_sb[:], emat[:])

        # PE warm-up: ~6us of dependency-free matmuls on a zeroed tile while
        # the input DMAs stream in. HAM un-throttles the PE clock (1.2 ->
        # 2.4 GHz) after ~3.4us of sustained activity; without this the whole
        # DMA-paced v phase runs at half clock.
        wu_sb = cst_pool.tile([128, 512], BF16, tag="wu")
        nc.gpsimd.memset(wu_sb[:], 0.0)
        for _ in range(16):
            ps_w = ps_pool.tile([128, 512], F32, tag="qkps", name="ps_qkps",
                                bufs=2)
            nc.tensor.matmul(ps_w[:], wu_sb[:, 0:128], wu_sb[:],
                             start=True, stop=True)

        # persistent tiles
        ut_sb = [ut_pool.tile([128, T], BF16, tag="ut", name=f"ut{i}")
                 for i in range(NEC)]
        dn_sb = dn_pool.tile([HL, T], F32, tag="dn")
        rd_sb = dn_pool.tile([HL, T], F32, tag="rd")
        rdb_sb = dn_pool.tile([HL, T], BF16, tag="rdb")
        v_sb = [vsb_pool.tile([128, VROW], BF16, tag="vsb", name=f"vsb{i}")
                for i in range(NTC)]
        # garbage rows of dn would hit reciprocal before they are written;
        # keep them finite so 0*inf NaNs can't leak out of the R matmul
        nc.gpsimd.memset(dn_sb[:], 1.0)

        wp_sb = []
        for ecn in range(NEC):
            t = wp_pool.tile([128, D], BF16, tag="wp", name=f"wpt{ecn}")
            nc.gpsimd.dma_start(t[:], wp[ecn * 128:(ecn + 1) * 128, :])
            wp_sb.append(t)

        def ps_tile(tag, bufs):
            return ps_pool.tile([128, 512], F32, tag=tag, name=f"ps_{tag}",
                                bufs=bufs)

        def proj_pass(ecs, out_t, dge, alt_from=None):
            """One projection pass accumulating a subset of e-chunks into
            its own partial output (summed on the host). Steps >= alt_from
            alternate between the two 512-wide PSUM tags for deeper
            buffering (only safe once the attention chunks no longer use
            the utps tiles)."""
            for tcn in range(NTC):
                for ob in range(2):
                    step = tcn * 2 + ob
                    if alt_from is not None and step >= alt_from and step % 2:
                        ps_p = ps_tile("utps", 2)
                    else:
                        ps_p = ps_tile("qkps", 2)
                    for i, ecn in enumerate(ecs):
                        nc.tensor.matmul(
                            ps_p[:],
                            ut_sb[ecn][:, tcn * 128:(tcn + 1) * 128],
                            wp_sb[ecn][:, ob * 512:(ob + 1) * 512],
                            start=(i == 0), stop=(i == len(ecs) - 1))
                    st = stage_pool.tile([128, 512], BF16, tag="st",
                                         name="stg")
                    nc.vector.tensor_copy(st[:], ps_p[:])
                    dge.dma_start(
                        out_t[tcn * 128:(tcn + 1) * 128,
                              ob * 512:(ob + 1) * 512], st[:])
                    yield

        # projA drips during chunks 2-3 (outA stores ride gpsimd so the
        # quiet Sync queue keeps the denominator repacks low-latency);
        # projB's t-blocks 0-1 drip into the tail of chunk 3 right after
        # their normalize, the rest interleaves with normalize(3)
        projA = proj_pass((0, 1), outA, nc.gpsimd)
        projB = proj_pass((2, 3), outB, nc.sync, alt_from=16)

        with tc.tile_pool(name="x8", bufs=NPAIR) as x8_pool:
            x8_sb = []
            qk_chunks = {}

            def pair3(t, free):
                """[128, 2*free] tile -> DoubleRow [K, 2, free] view."""
                return t[:].rearrange("p (two f) -> p two f", two=2)

            def qk_filler(ec):
                """Generator computing q/k chunks for `ec` via DoubleRow fp8
                matmuls; yields between small PE steps so it can be dripped
                into the attention loop as filler work. kT is stored twice,
                zero-padded per head parity, so the score matmuls run with
                K=128."""
                wq_t, wk_t = [], []
                for p in range(NPAIR):
                    t = wqk_pool.tile([128, 256], FP8, tag="wqk", name="wqkt")
                    nc.gpsimd.dma_start(
                        t[:], wq8[p * 128:(p + 1) * 128,
                                  ec * 256:(ec + 1) * 256])
                    wq_t.append(t)
                for p in range(NPAIR):
                    t = wqk_pool.tile([128, 256], FP8, tag="wqk", name="wqkt")
                    nc.gpsimd.dma_start(
                        t[:], wk8[p * 128:(p + 1) * 128,
                                  ec * 256:(ec + 1) * 256])
                    wk_t.append(t)
                q_ec = qk_pool.tile([128, T], BF16, tag="qk", name="q_ec")
                kA = qk_pool.tile([128, T], BF16, tag="qk", name="kA")
                kB = qk_pool.tile([128, T], BF16, tag="qk", name="kB")
                nc.gpsimd.memset(kA[64:128, :], 0.0)
                nc.gpsimd.memset(kB[0:64, :], 0.0)
                qk_chunks[ec] = (q_ec, kA, kB)
                # tbp-major so the first half of the steps only touch the
                # low-t halves of x8 (which land first)
                for tbp in range(2):
                    for (w_t, iskA) in ((wq_t, False), (wk_t, True)):
                        pss = [ps_tile("qkps", 2) for _ in range(2)]
                        for p in range(NPAIR):
                            for i in range(2):
                                tb = 2 * tbp + i
                                nc.tensor.matmul(
                                    pss[i], pair3(w_t[p], 128),
                                    pair3(x8_sb[p], T)[
                                        :, :, tb * 512:(tb + 1) * 512],
                                    start=(p == 0), stop=(p == NPAIR - 1),
                                    perf_mode=DR)
                            yield
                        for i in range(2):
                            tb = 2 * tbp + i
                            sl = slice(tb * 512, (tb + 1) * 512)
                            if iskA:
                                nc.vector.tensor_copy(
                                    kA[0:64, sl], pss[i][0:64, :])
                                nc.vector.tensor_copy(
                                    kB[64:128, sl], pss[i][64:128, :])
                            else:
                                nc.vector.tensor_copy(q_ec[:, sl], pss[i][:])
                        yield

            def normalize(ec, tbs=None):
                """affinT = UT * 1/denom for chunk ec, sliced per t-block so
                the recip -> broadcast -> scale chain pipelines."""
                for tb in (range(NTB) if tbs is None else tbs):
                    sl = slice(tb * 512, (tb + 1) * 512)
                    nc.vector.reciprocal_approx_fast(rd_sb[:, sl],
                                                     dn_sb[:, sl])
                    nc.vector.tensor_copy(rdb_sb[:, sl], rd_sb[:, sl])
                    ps_r = ps_tile("qkps", 2)
                    nc.tensor.matmul(
                        ps_r[:], em_sb[:, ec * 128:(ec + 1) * 128],
                        rdb_sb[:, sl], start=True, stop=True)
                    nc.vector.tensor_mul(
                        ut_sb[ec][:, sl], ut_sb[ec][:, sl], ps_r[:])

            # ------------- phase A0: v = x @ Wv (+ dripped qk(0)) -------------
            with tc.tile_pool(name="wv", bufs=NDC) as wv_pool, \
                 tc.tile_pool(name="xt", bufs=NDC) as xt_pool:
                # xT and x8 stream in on the scalar engine's DGE queue so
                # their dispatches don't serialize behind wv/weights on
                # Sync; low-t halves of every chunk land first so the
                # dc-ordered accumulations can finish low-t blocks early
                wv_sb, xt_sb = [], []
                # x8 low halves first: they feed the filler steps that keep
                # the PE busy while the bigger xT stream lands
                for p in range(NPAIR):
                    t = x8_pool.tile([128, 2 * T], FP8, tag="x8",
                                     name=f"x8{p}")
                    nc.scalar.dma_start(t[:, 0:T // 2],
                                        x8[p * 128:(p + 1) * 128, 0:T // 2])
                    nc.scalar.dma_start(
                        t[:, T:T + T // 2],
                        x8[p * 128:(p + 1) * 128, T:T + T // 2])
                    x8_sb.append(t)
                for dc in range(NDC):
                    t = wv_pool.tile([128, DL], BF16, tag="wv", name=f"wv{dc}")
                    nc.sync.dma_start(t[:], wv[dc * 128:(dc + 1) * 128, :])
                    wv_sb.append(t)
                    t = xt_pool.tile([128, T], BF16, tag="xt", name=f"xt{dc}")
                    nc.scalar.dma_start(t[:, 0:T // 2],
                                        xT[dc * 128:(dc + 1) * 128, 0:T // 2])
                    xt_sb.append(t)
                for p in range(NPAIR):
                    nc.scalar.dma_start(
                        x8_sb[p][:, T // 2:T],
                        x8[p * 128:(p + 1) * 128, T // 2:T])
                    nc.scalar.dma_start(
                        x8_sb[p][:, T + T // 2:2 * T],
                        x8[p * 128:(p + 1) * 128, T + T // 2:2 * T])
                for dc in range(NDC):
                    nc.scalar.dma_start(
                        xt_sb[dc][:, T // 2:T],
                        xT[dc * 128:(dc + 1) * 128, T // 2:T])

                filler0 = qk_filler(0)
                # lead with filler steps that only need the x8 low halves:
                # they bridge the PE from the warm-up to the first v matmul
                # while wv/xT are still streaming in
                for _ in range(8):
                    next(filler0, None)
                for tcn in range(NTC):
                    ps_v = ps_tile("utps", 2)
                    for dc in range(NDC):
                        nc.tensor.matmul(
                            ps_v[:], xt_sb[dc][:, tcn * 128:(tcn + 1) * 128],
                            wv_sb[dc][:], start=(dc == 0),
                            stop=(dc == NDC - 1))
                    dst = v_sb[tcn][:].rearrange("p (e c) -> p e c", c=VPAIR)
                    src = ps_v[:].rearrange("p (e c) -> p e c", c=128)
                    nc.vector.tensor_copy(dst[:, :, 0:64], src[:, :, 0:64])
                    nc.vector.tensor_copy(dst[:, :, 128:192], src[:, :, 64:128])
                    nc.gpsimd.memset(dst[:, :, 64:65], 1.0)
                    nc.gpsimd.memset(dst[:, :, 65:128], 0.0)
                    next(filler0, None)
                for _ in filler0:
                    pass
            # wv + xt pools released here (fillers stream x8, not xT)

            # ------------- per e-chunk: attention + dripped filler work -------------
            def attention_chunk(ec, drip):
                q_ec, kA, kB = qk_chunks.pop(ec)
                slot = [0]
                for par in range(2):       # head parity within chunk
                    h = 2 * ec + par       # local head index
                    kpad = kA if par == 0 else kB
                    for ibp in range(2):   # i-block pair (2*ibp, 2*ibp+1)
                        ibl, ibr = 2 * ibp, 2 * ibp + 1
                        utl = ps_tile("utps", 2)
                        utr = ps_tile("utps", 2)
                        for jt in range(4 * ibr + 4):
                            drip(slot[0])
                            slot[0] += 1
                            dl = (jt // 4 == ibl)
                            skip_l = (jt // 4 > ibl)
                            dr = (jt // 4 == ibr)
                            cl = 128 * (jt - 4 * ibl) if dl else 0
                            cr = 128 * (jt - 4 * ibr) if dr else 0
                            c0 = 512 + cr if skip_l else cl
                            st_ps = ps_pool.tile([128, 1024], F32, tag="stps",
                                                 name="ps_stps", bufs=2)
                            kh_j = kpad[:, jt * 128:(jt + 1) * 128]
                            if not skip_l:
                                nc.tensor.matmul(
                                    st_ps[:, cl:512], kh_j,
                                    q_ec[:, ibl * 512 + cl:(ibl + 1) * 512],
                                    start=True, stop=True)
                            nc.tensor.matmul(
                                st_ps[:, 512 + cr:1024], kh_j,
                                q_ec[:, ibr * 512 + cr:(ibr + 1) * 512],
                                start=True, stop=True)
                            pt_t = pt_pool.tile([128, 1024], BF16, tag="pt")
                            nc.scalar.activation(
                                pt_t[:, c0:1024], st_ps[:, c0:1024], EXP,
                                scale=SCALE8)
                            if dl:
                                nc.vector.tensor_mul(
                                    pt_t[:, cl:cl + 128],
                                    pt_t[:, cl:cl + 128], mk_sb[:])
                            if dr:
                                nc.vector.tensor_mul(
                                    pt_t[:, 512 + cr:512 + cr + 128],
                                    pt_t[:, 512 + cr:512 + cr + 128], mk_sb[:])
                            # PV: [v|1].T @ PT -> UT rows + denom row
                            vt = v_sb[jt][:].rearrange(
                                "p (e c) -> p e c", c=VPAIR)[:, ec, :]
                            if par == 0:
                                lhs = vt[:, 0:65]       # M=65 -> rows 0..64
                                rsl = slice(0, 65)
                            else:
                                # [one|junk63|v_odd]: denom row 0, v 64..127
                                lhs = vt[:, 64:192]     # M=128
                                rsl = slice(0, 128)
                            if not skip_l:
                                nc.tensor.matmul(
                                    utl[rsl, cl:512], lhs, pt_t[:, cl:512],
                                    start=(jt == 0), stop=(jt == 4 * ibl + 3),
                                    skip_group_check=True)
                            nc.tensor.matmul(
                                utr[rsl, cr:512], lhs, pt_t[:, 512 + cr:1024],
                                start=(jt == 0), stop=(jt == 4 * ibr + 3),
                                skip_group_check=True)
                            for ib_d, ut_d in ((ibl, utl), (ibr, utr)):
                                if jt != 4 * ib_d + 3:
                                    continue
                                if par == 0:
                                    usrc, dsrc, r = (ut_d[0:64, :],
                                                     ut_d[64:65, :], 64)
                                    udst = ut_sb[ec][
                                        0:64, ib_d * 512:(ib_d + 1) * 512]
                                else:
                                    usrc, dsrc, r = (ut_d[64:128, :],
                                                     ut_d[0:1, :], 0)
                                    udst = ut_sb[ec][
                                        64:128, ib_d * 512:(ib_d + 1) * 512]
                                with tc.high_priority():
                                    nc.vector.tensor_copy(udst, usrc)
                                    # denom: same-partition copy + DMA repack
                                    stg = dns_pool.tile([128, 512], F32,
                                                        tag="dns",
                                                        name="dnstg")
                                    nc.vector.tensor_copy(stg[r:r + 1, :],
                                                          dsrc)
                                nc.sync.dma_start(
                                    dn_sb[h:h + 1,
                                          ib_d * 512:(ib_d + 1) * 512],
                                    stg[r:r + 1, :])

            # The three remaining fillers (20 PE steps each) feed one shared
            # work stream dripped across chunks 0-2; pacing keeps each
            # filler comfortably ahead of the chunk that consumes it
            # (f1 by mid-c0, f2 by mid-c1, f3 by early c2) while leaving
            # every chunk enough PE drip work to bridge the exp latency.
            from itertools import chain as _chain
            fillers = _chain(qk_filler(1), qk_filler(2), qk_filler(3))

            for ec in range(NEC - 1):

                def drip(slot, ec=ec):
                    if ec < 2:
                        if slot % 8 in (0, 2, 3, 5):      # 24 per chunk
                            next(fillers, None)
                        elif ec > 0 and slot == 6:
                            normalize(ec - 1)
                    else:
                        if slot % 12 in (0, 3, 5, 8, 10):  # f3 done by ~s28
                            if next(fillers, None) is None and slot >= 9:
                                next(projA, None)
                        elif slot == 6:
                            normalize(ec - 1)
                        elif slot >= 9:
                            next(projA, None)

                attention_chunk(ec, drip)
                if ec == 2:
                    for _ in fillers:   # safety drain (normally empty)
                        pass
        # x8 pool released here (before the last attention chunk)

        def drip3(slot):
            if slot == 6:
                normalize(NEC - 2)
            elif slot == 29:
                normalize(NEC - 1, [0])
            elif slot == 33:
                normalize(NEC - 1, [1])
            elif slot >= 34:
                next(projB, None)
            elif slot != 0:
                next(projA, None)

        attention_chunk(NEC - 1, drip3)
        for _ in projA:
            pass
        for _ in range(2):           # finish t-block 1 (ready)
            next(projB, None)
        normalize(NEC - 1, [2])
        for _ in range(8):           # t-block 2
            next(projB, None)
        normalize(NEC - 1, [3])
        for _ in projB:              # t-block 3
            pass



_NC_CACHE = None


def _get_nc():
    global _NC_CACHE
    if _NC_CACHE is None:
        _NC_CACHE = _build()
    return _NC_CACHE


def make_in_maps(embds, W_qkv, W_proj):
    embds = np.asarray(embds, dtype=np.float32)
    W_qkv = np.asarray(W_qkv, dtype=np.float32)
    W_proj = np.asarray(W_proj, dtype=np.float32)
    E4 = ml_dtypes.float8_e4m3

    mask_np = np.triu(np.ones((128, 128))).astype(ml_dtypes.bfloat16)
    emat_np = np.kron(np.eye(HL), np.ones((1, Dh))).astype(ml_dtypes.bfloat16)

    def pack_w(w):
        # [1024, 512] -> [512, 1024]: row pair*128+r, col ec*256+two*128+c
        return np.ascontiguousarray(
            (WS * w).reshape(NPAIR, 2, 128, NEC, 128)
            .transpose(0, 2, 3, 1, 4).reshape(DL, 2 * DL)).astype(E4)

    in_maps = []
    for c in range(NCORES):
        b, hg = c // 2, c % 2
        sl = slice(hg * DL, (hg + 1) * DL)
        xT = np.ascontiguousarray(embds[b].T)
        x8 = np.ascontiguousarray(
            (XS * xT).reshape(NPAIR, 2, 128, T)
            .transpose(0, 2, 1, 3).reshape(DL, 2 * T)).astype(E4)
        in_maps.append({
            "x8": x8,
            "wk8": pack_w(W_qkv[:, 0 * D:1 * D][:, sl]),
            "wq8": pack_w(W_qkv[:, 1 * D:2 * D][:, sl]),
            "xT": xT.astype(ml_dtypes.bfloat16),
            "wv": np.ascontiguousarray(
                W_qkv[:, 2 * D:3 * D][:, sl]).astype(ml_dtypes.bfloat16),
            "wp": np.ascontiguousarray(
                W_proj[sl, :]).astype(ml_dtypes.bfloat16),
            "mask": mask_np,
            "emat": emat_np,
        })
    return in_maps


def gather_out(results, b_proj):
    b_proj = np.asarray(b_proj, dtype=np.float32)
    full = np.empty((B, T, D), dtype=np.float32)
    for b in range(B):
        acc = np.zeros((T, D), dtype=np.float32)
        for c in (2 * b, 2 * b + 1):
            acc += np.asarray(results[c]["outA"], dtype=np.float32)
            acc += np.asarray(results[c]["outB"], dtype=np.float32)
        full[b] = acc + b_proj[None, :]
    return full


def kernel(embds, W_qkv, W_proj, b_proj):
    in_maps = make_in_maps(embds, W_qkv, W_proj)
    nc = _get_nc()
    res = run_bass_kernel_spmd(nc, in_maps, list(range(NCORES)))
    return gather_out(res.results, b_proj)
